# revision 68
# baseline (speedup 1.0000x reference)
"""Trainium2 Bass kernel for nn_ContrastiveCriterion.

Reference semantics (per sample b of B=2, N=4096, D=512):
    refer = l2_normalize(emb_point[b][pos_idx[b]])      # [N, D]
    key   = l2_normalize(emb_text[b])                   # [N, D]
    sim   = refer @ key.T                               # [N, N]
    ce_p[i] = logsumexp_j(ls*sim[i,j]) - ls*sim[i,i]
    ce_t[j] = logsumexp_i(ls*sim[i,j]) - ls*sim[j,j]
    loss_b  = mean_i(0.5*(ce_p+ce_t)*dist_norm[b])
    rank_b  = sum_ij relu(sim[i,j] - sim[j,j])
    out = (mean_b loss_b, 0.5 * mean_b rank_b)

Device strategy (8 cores = 2 samples x 4 row-chunks of 1024): each core
computes its [1024, 4096] row-stripe of sim ONCE, in fp8 (DoubleRow).
Per [128, 1024] psum tile:
  - ACT: exp(ls*sim) with accum_out -> row-sums (sp partials), esc in bf16
  - PE:  ones^T @ esc accumulated over the 8 row-tiles -> column-sums of
         exp (st partials), i.e. the transposed-direction logsumexp sums
  - DVE: tensor_tensor_reduce max(sim, d_j) + row-sum -> rank partials,
         using sum_j relu(s-d) = sum_j max(s,d) - sum_j d (host corrects)
The host does gather, l2-normalization, fp8 quantization (x16 to dodge
the subnormal tail; psum holds 256*sim), the diagonal d = rowdot(refer_n,
key_n), and the final O(N) f64 reductions.
"""

import numpy as np
import ml_dtypes

import concourse.bass as bass
import concourse.tile as tile
import concourse.mybir as mybir
from concourse.bass_utils import run_bass_kernel_spmd

B, N, D = 2, 4096, 512
P = 128                 # SBUF partitions
KC = D // P             # 4 contraction chunks of 128
QPER = 4                # cores per sample
CHUNK = N // QPER       # 1024 rows per core
TI = CHUNK // P         # 8 row tiles per core
JQ = 4                  # column quarters
QW = N // JQ            # 1024 cols per quarter
# column parts per row-stripe: four 1024-wide psum tiles
PARTS = ((0, QW), (QW, QW), (2 * QW, QW), (3 * QW, QW))
NTL = TI * len(PARTS)   # 32 tiles per core
NCH = N // P            # 32 column chunks (st layout)
SCALE = 16.0            # fp8 pre-scale; psum sim values carry SCALE^2

bf16 = mybir.dt.bfloat16
f32 = mybir.dt.float32
f8 = mybir.dt.float8e4

# set by kernel() for test harness introspection
LAST_RESULT = None

MAX_DRAIN_WAITS = 1


def _parallelize_ps_readers(nc: bass.Bass) -> None:
    """Let exp(k) run concurrently with STT(k).

    Tile's per-tile last-user chain serializes the two readers of each psum
    tile (STT first, then exp waits STT via a DVE-sem wait), which delays
    the whole ACT chain by one STT.  The ordering the wait actually protects
    is the WAR against mains(k+3) reusing the psum buffer (bufs=3) — and
    mains(k+3) already covers exp(k) transitively through its PE wait
    (colsum(k) >= exp(k)).  So move exp(k)'s DVE wait (value = STT(k)
    completion) onto the first main matmul of tile k+3, where it expresses
    the real dependency, and let exp start as soon as its mains finish.
    """
    acts = []     # exp instructions in emission order
    cols1 = []    # first matmul of each colsum group (non-DoubleRow)
    cs_count = 0
    for fn in nc.m.functions:
        for bb in fn.blocks:
            for ins in bb.instructions:
                if (
                    isinstance(ins, mybir.InstActivation)
                    and ins.engine == mybir.EngineType.Activation
                    and ins.sync_info is not None
                    and ins.sync_info.on_wait
                    and any(w.ant_name.startswith("DVE") and w.wait_value >= 2
                            for w in ins.sync_info.on_wait)
                ):
                    acts.append(ins)
                if (
                    isinstance(ins, mybir.InstMatmult)
                    and ins.perf_mode is None
                    and ins.outs and "acc2" in str(ins.outs[0].memref)
                ):
                    if cs_count % (QW // P) == 0:
                        cols1.append(ins)
                    cs_count += 1
    for j, exp in enumerate(acts):
        waits = list(exp.sync_info.on_wait)
        dve = [w for w in waits if w.ant_name.startswith("DVE")]
        if not dve:
            continue
        if j >= len(cols1):
            continue
        exp.sync_info.on_wait = [w for w in waits if w not in dve]
        tgt = cols1[j]
        if tgt.sync_info is None:
            tgt.sync_info = mybir.SyncInfo(on_wait=[], on_update=[])
        tgt.sync_info.on_wait = list(tgt.sync_info.on_wait or []) + dve


def _split_drain_waits(nc: bass.Bass, max_waits: int = MAX_DRAIN_WAITS) -> None:
    # walrus codegen for TRN2 CTRL instructions (Drain) accepts a limited
    # number of sync-wait slots; split over-limit drains into a chain.
    for fn in nc.m.functions:
        for bb in fn.blocks:
            insts = list(bb.instructions)
            out, n_extra = [], 0
            for ins in insts:
                si = ins.sync_info
                if si is not None and si.on_wait and len(si.on_wait) > max_waits:
                    waits = list(si.on_wait)
                    for k in range(0, len(waits) - max_waits, max_waits):
                        extra = mybir.InstDrain(
                            name=f"{ins.name}_prewait{k}",
                            ins=[],
                            outs=[],
                        )
                        extra.engine = ins.engine
                        extra.sync_info = mybir.SyncInfo(
                            on_wait=waits[k: k + max_waits], on_update=[]
                        )
                        out.append(extra)
                        n_extra += 1
                    si.on_wait = waits[len(waits) - max_waits:]
                out.append(ins)
            if n_extra:
                bb.instructions[:] = out

    # kernel-tail drain chain: the last block serializes one 100ns drain per
    # DMA-completion sem on SP.  Spread them across the other engines — the
    # final all-engine barrier joins every stream, so any engine may carry
    # the wait.
    if nc.m.functions:
        last_bb = nc.m.functions[0].blocks[-1]
        targets = [mybir.EngineType.PE, mybir.EngineType.DVE,
                   mybir.EngineType.Activation, mybir.EngineType.Pool]
        ti = 0
        for ins in last_bb.instructions:
            si = ins.sync_info
            if (
                isinstance(ins, mybir.InstDrain)
                and ins.engine == mybir.EngineType.SP
                and si is not None and si.on_wait
                and all(w.ant_name.startswith("DMA") for w in si.on_wait)
            ):
                ins.engine = targets[ti % len(targets)]
                ti += 1


def build_program(logit_scale: float) -> bass.Bass:
    nc = bass.Bass()

    pt = nc.declare_dram_parameter("pt", [P, KC, CHUNK], f8, isOutput=False)
    tx = nc.declare_dram_parameter("tx", [P, KC, N], f8, isOutput=False)
    dbc = nc.declare_dram_parameter("dbc", [P, N], bf16, isOutput=False)
    # out_a packs sp (cols 0:NTL) and st (cols NTL:NTL+32) in one buffer/DMA
    out_a = nc.declare_dram_parameter("out_a", [P, NTL + NCH], f32, isOutput=True)
    out_r = nc.declare_dram_parameter("out_r", [P, NTL], f32, isOutput=True)

    Act = mybir.ActivationFunctionType
    Alu = mybir.AluOpType
    DR = mybir.MatmulPerfMode.DoubleRow
    inv_s2 = 1.0 / (SCALE * SCALE)

    with tile.TileContext(nc) as tc:
        with tc.tile_pool(name="main", bufs=1) as pmain:
            pts = pmain.tile([P, KC, CHUNK], f8, name="pts", tag="pts")
            txs = pmain.tile([P, KC, N], f8, name="txs", tag="txs")
            dbs = pmain.tile([P, N], bf16, name="dbs", tag="dbs")
            ones_t = pmain.tile([P, P], bf16, name="ones_t", tag="ones_t")
            a_parts = pmain.tile([P, NTL + NCH], f32, name="a_parts", tag="a_parts")
            r_parts = pmain.tile([P, NTL], f32, name="r_parts", tag="r_parts")
            sp_parts = a_parts[:, 0:NTL]
            st_sb = a_parts[:, NTL:NTL + NCH]

            nc.vector.memset(ones_t, 1.0)

            # loads: tx columns [0:1536) (sync queue) and pts (ACT queue,
            # idle during the preamble) gate the first tiles; rest streams
            W0 = PARTS[0][1]
            nc.sync.dma_start(
                out=txs[:, :, 0:W0],
                in_=tx[:, :, 0:W0],
            )
            nc.scalar.dma_start(out=pts[:, :, 0:P], in_=pt[:, :, 0:P])
            nc.scalar.dma_start(out=pts[:, :, P:CHUNK], in_=pt[:, :, P:CHUNK])
            nc.sync.dma_start(
                out=txs[:, :, W0:N],
                in_=tx[:, :, W0:N],
            )

            # preload the Exp activation table while DMAs run
            warm = pmain.tile([P, 1], bf16, name="warm", tag="warm")
            nc.scalar.activation(warm, ones_t[:, 0:1], Act.Exp, scale=1.0)
            nc.gpsimd.dma_start(out=dbs[:, 0:W0], in_=dbc[:, 0:W0])
            nc.gpsimd.dma_start(out=dbs[:, W0:N], in_=dbc[:, W0:N])

            with tc.tile_pool(name="scr", bufs=2) as pscr:
                with tc.tile_pool(name="psmm", bufs=1, space="PSUM") as ppm:
                    # part-major tile order: all row-stripes of columns
                    # [0:1536), then [1536:3072), then [3072:4096) — later
                    # parts' tx/dbs DMAs stream in behind the compute
                    tiles = [(ti, c0, w) for (c0, w) in PARTS for ti in range(TI)]
                    # st accumulator: one psum bank; column c0//128+cc holds
                    # sum_i esc[i, c0 + cc*128 + p] accumulated across tiles
                    acc2 = ppm.tile([P, NCH], f32, name="acc2", tag="acc2", bufs=1)
                    # rank scratch: WAW-only on DVE
                    mxs = pmain.tile([P, PARTS[0][1]], bf16, name="mxs", tag="mxs")

                    def emit_act_dve(k, ti, c0, w, ps):
                        # rank: sum_j max(256*sim, 256*d_j); host divides 256.
                        # (tensor_tensor_reduce is unsupported by walrus
                        # codegen — scalar_tensor_tensor is equivalent here)
                        nc.vector.scalar_tensor_tensor(
                            out=mxs[:, 0:w], in0=ps[:, 0:w], scalar=1.0,
                            in1=dbs[:, c0:c0 + w],
                            op0=Alu.mult, op1=Alu.max,
                            accum_out=r_parts[:, k: k + 1],
                        )
                        # unique esc buffer per tile: exp never WARs a prior
                        # colsum read, so its only sem wait is its mains
                        esc = pscr.tile([P, w], bf16, name=f"esc_{k}",
                                        tag=f"esc_{k}", bufs=1)
                        nc.scalar.activation(
                            esc, ps[:, 0:w], Act.Exp,
                            scale=float(logit_scale) * inv_s2,
                            accum_out=sp_parts[:, k: k + 1],
                        )
                        if k == NTL // 2 - 1:
                            # first half of the rank partials can ship early
                            nc.sync.dma_start(
                                out=out_r[:, 0:NTL // 2],
                                in_=r_parts[:, 0:NTL // 2],
                            )
                        return esc

                    def emit_colsum(k, ti, c0, w, esc):
                        # column-sums of exp: esc chunk as stationary operand
                        # against a ones column vector, accumulated over ti.
                        # start only on the very first matmul: start zeroes
                        # the whole 2KB zero-region (bank), so later columns'
                        # first writes auto-zero; restarting per column would
                        # re-mark the bank and clobber earlier columns.
                        ch0 = c0 // P
                        for cc in range(w // P):
                            nc.tensor.matmul(
                                acc2[:, ch0 + cc: ch0 + cc + 1],
                                lhsT=esc[:, cc * P:(cc + 1) * P],
                                rhs=ones_t[:, 0:1],
                                start=(k == 0 and cc == 0),
                                stop=(k == NTL - 1 and cc == w // P - 1),
                                skip_group_check=True,
                            )

                    # Tile sem waits are engine-count-at-emission, so every
                    # cross-engine "emitted after" edge becomes a wait.  To
                    # give all waits slack, mains run two tiles ahead:
                    #   prologue: mains(0), mains(1)
                    #   iter k:   STT/exp(k); colsum(k-2); mains(k+2)
                    pss = {}

                    def emit_mains(k):
                        ti, c0, w = tiles[k]
                        ps = ppm.tile([P, PARTS[0][1]], f32, name=f"ps_{k}",
                                      tag="mm", bufs=3)
                        for kg in range(KC // 2):
                            for blk in range(w // 512):
                                nc.tensor.matmul(
                                    ps[:, blk * 512:(blk + 1) * 512],
                                    lhsT=pts[:, 2 * kg:2 * kg + 2, ti * P:(ti + 1) * P],
                                    rhs=txs[:, 2 * kg:2 * kg + 2,
                                            c0 + blk * 512: c0 + (blk + 1) * 512],
                                    start=(kg == 0),
                                    stop=(kg == KC // 2 - 1),
                                    perf_mode=DR,
                                )
                        pss[k] = ps

                    emit_mains(0)
                    emit_mains(1)
                    escs = {}
                    for k, (ti, c0, w) in enumerate(tiles):
                        escs[k] = emit_act_dve(k, ti, c0, w, pss.pop(k))
                        if k >= 2:
                            kc_ = k - 2
                            tic, c0c, wc = tiles[kc_]
                            emit_colsum(kc_, tic, c0c, wc, escs[kc_])
                        if k + 2 < NTL:
                            emit_mains(k + 2)
                    # second rank half can ship as soon as the last STT is
                    # done, overlapping the final colsums and st copy
                    nc.sync.dma_start(
                        out=out_r[:, NTL // 2:], in_=r_parts[:, NTL // 2:]
                    )
                    for kc_ in (NTL - 2, NTL - 1):
                        tic, c0c, wc = tiles[kc_]
                        emit_colsum(kc_, tic, c0c, wc, escs[kc_])
                    nc.scalar.copy(st_sb, acc2)

            nc.scalar.dma_start(out=out_a[:, :], in_=a_parts)

    # NOTE: _parallelize_ps_readers (letting exp(k) run concurrently with
    # STT(k)) gains ~2.5us in CoreSim but fails on hardware — the PE wait
    # queue lets ready matmuls bypass a waiting colsum, so the psum WAR
    # protection cannot be anchored on PE program order.  Left disabled.
    _split_drain_waits(nc)
    return nc


def _normalize(x: np.ndarray) -> np.ndarray:
    n = np.linalg.norm(x, axis=-1, keepdims=True)
    return x / np.maximum(n, 1e-12)


def _prep_sample(emb_point, emb_text, pos_idx, b):
    """gather + l2-normalize + diagonal for sample b (host, f32)."""
    g = np.asarray(emb_point[b])[np.asarray(pos_idx[b])]
    gn = _normalize(g.astype(np.float32))
    kn = _normalize(np.asarray(emb_text[b]).astype(np.float32))
    d = np.einsum("nd,nd->n", gn, kn)
    return gn, kn, d


def _pack_T(x: np.ndarray) -> np.ndarray:
    """[rows, D] f32 -> [128, KC, rows] fp8 with k = c*128 + p."""
    rows = x.shape[0]
    xt = np.ascontiguousarray(
        (x * SCALE).T.reshape(KC, P, rows).transpose(1, 0, 2)
    )
    return xt.astype(ml_dtypes.float8_e4m3)


def make_in_maps(inputs: dict) -> tuple[list[dict], list]:
    samples = [
        _prep_sample(inputs["emb_point"], inputs["emb_text"], inputs["pos_idx"], b)
        for b in range(B)
    ]
    in_maps = []
    for core in range(2 * QPER):
        b, q = core // QPER, core % QPER
        gn, kn, d = samples[b]
        c0 = q * CHUNK
        drot = np.roll(d, -c0)
        dbc = np.ascontiguousarray(
            np.broadcast_to(
                (drot * SCALE * SCALE).astype(ml_dtypes.bfloat16)[None, :], (P, N)
            )
        )
        in_maps.append({
            "pt": _pack_T(gn[c0:c0 + CHUNK]),
            "tx": _pack_T(np.roll(kn, -c0, axis=0)),
            "dbc": dbc,
        })
    return in_maps, samples


def postprocess(results, samples, dist_norm, ls: float):
    losses, ranks = [], []
    for b in range(B):
        _, _, d = samples[b]
        d64 = d.astype(np.float64)
        sp = np.empty(N, np.float64)
        st = np.zeros(N, np.float64)
        rank_b = 0.0
        for q in range(QPER):
            r = results[b * QPER + q]
            c0 = q * CHUNK
            # sp: rows [c0, c0+1024); op k=part*TI+ti covers rows ti*128+p
            parts = r["out_a"][:, 0:NTL].astype(np.float64).reshape(P, len(PARTS), TI)
            sp_loc = parts.sum(axis=1)          # [P, TI]
            sp[c0:c0 + CHUNK] = sp_loc.T.reshape(-1)
            # st: st_sb[p, ch] covers local col ch*128 + p;
            # local col j -> global (c0 + j) % N
            st_loc = (
                r["out_a"][:, NTL:].astype(np.float64)
                .reshape(P, NCH).T.reshape(-1)
            )
            st += np.roll(st_loc, c0)
            # rank: sum of max(256*sim, 256*d_j)/256 minus 1024 * sum_j d_j
            rank_b += r["out_r"].astype(np.float64).sum() / (SCALE * SCALE)
            rank_b -= CHUNK * d64.sum()
        ce_p = np.log(sp) - ls * d64
        ce_t = np.log(st) - ls * d64
        dn = np.asarray(dist_norm[b], dtype=np.float64)
        losses.append(np.mean(0.5 * (ce_p + ce_t) * dn))
        ranks.append(rank_b)
    contrastive = np.float32(np.mean(losses))
    rank_loss = np.float32(0.5 * np.mean(ranks))
    return contrastive, rank_loss


def kernel(emb_point, emb_text, dist_norm, pos_idx, logit_scale):
    global LAST_RESULT
    import os

    ls = float(np.asarray(logit_scale, dtype=np.float64).reshape(-1)[0])
    nc = build_program(ls)

    inputs = {"emb_point": emb_point, "emb_text": emb_text, "pos_idx": pos_idx}
    in_maps, samples = make_in_maps(inputs)

    trace = bool(int(os.environ.get("KERNEL_TRACE", "0")))
    res = run_bass_kernel_spmd(nc, in_maps, list(range(2 * QPER)), trace=trace)
    LAST_RESULT = res

    return postprocess(res.results, samples, dist_norm, ls)
